# revision 2
# baseline (speedup 1.0000x reference)
"""DKVMN knowledge-tracing model on 8 Trainium2 NeuronCores — v2.

Sharding: data-parallel over batch (B=32 -> 4 rows/core); params replicated.

Per-core algorithm (BL=4, T=512, D=128, M=50), fp16 data / f32 accumulation:
  phase A: e = sigmoid(We v), a = tanh(Wa v), w = softmax_m(k @ Mk^T)
           (k/v arrive pre-gathered + transposed as fp16 [D, BL*T] from host)
  phase B (per m): one fused scan over all 4 batch rows using a
           W = BL*(T+1) = 2052-wide layout with a reset column per row
           (A=0, B=Mv0 at col b*513 restarts the recurrence at Mv0).
           Engine split per m:
             PE:   broadcast w[m] to 128 partitions (2 matmuls -> PSUM halves)
                   + 4 accumulate matmuls for reads
             Act:  PSUM->SBUF fp16 copy of the broadcast; A = 1 - w*e
                   (activation scale=-1 bias=1); Mv0 boundary patch
             DVE:  we = wbc*e, Bt = wbc*a (fp16 2x); the tensor_tensor_scan
             Pool: rt = wbc*S_pre via scalar_tensor_tensor (gpsimd)
  phase C: f = tanh(Wf [reads;k]); p = sigmoid(Wp f)
"""

import numpy as np
from contextlib import ExitStack

import concourse.bass as bass
import concourse.mybir as mybir
from concourse import tile
from concourse.bass_utils import run_bass_kernel_spmd
from concourse import bacc

B, T, D, M, NQ = 32, 512, 128, 50, 1000
NCORES = 8
BL = B // NCORES          # 4 batch rows per core
BT = BL * T               # 2048
W = BL * (T + 1)          # 2052: per-row segment of 513 (reset col + 512)
SEG = T + 1               # 513
F32 = mybir.dt.float32
F16 = mybir.dt.float16

# fp16 param pack layout (columns of prm16 [D, NP16])
C_IDEN = 0                # [D, 128] identity (racc lhsT; cols 0..49 also sel_m)
C_WET = 128               # We^T
C_WAT = 256               # Wa^T
C_WFRT = 384              # Wf[:, :D]^T
C_WFKT = 512              # Wf[:, D:]^T
C_MKT = 640               # Mk^T   [D, 50]
C_MV0 = 690               # Mv0^T  [D, 50]
C_WPT = 740               # Wp^T   [D, 1]
C_ONE = 741               # ones   [D, 1]
NP16 = 742

_CACHE = {}


def _ap3(t, col, s1, n1, s2, n2):
    """3-dim AP into tile t: partitions x [s1,n1] x [s2,n2], at column col."""
    base = t[:, col : col + 1]
    return bass.AP(base.tensor, base.offset, [list(base.ap[0]), [s1, n1], [s2, n2]])


def _build():
    nc = bacc.Bacc("TRN2", target_bir_lowering=False)

    kT = nc.dram_tensor("kT", [D, BT], F16, kind="ExternalInput")
    vT = nc.dram_tensor("vT", [D, BT], F16, kind="ExternalInput")
    prm16 = nc.dram_tensor("prm16", [D, NP16], F16, kind="ExternalInput")
    prm32 = nc.dram_tensor("prm32", [D, 4], F32, kind="ExternalInput")
    out = nc.dram_tensor("out", [1, BT], F32, kind="ExternalOutput")

    mult = mybir.AluOpType.mult
    add = mybir.AluOpType.add
    ACT = mybir.ActivationFunctionType

    with tile.TileContext(nc) as tc, ExitStack() as ctx:
        const = ctx.enter_context(tc.tile_pool(name="const", bufs=1))
        big = ctx.enter_context(tc.tile_pool(name="big", bufs=1))
        sp = ctx.enter_context(tc.tile_pool(name="sp", bufs=3))

        kT_s = const.tile_from(kT[:])
        vT_s = const.tile_from(vT[:])
        p16 = const.tile_from(prm16[:])
        p32 = const.tile_from(prm32[:])
        iden_s = p16[:, C_IDEN : C_IDEN + 128]
        WeT_s = p16[:, C_WET : C_WET + 128]
        WaT_s = p16[:, C_WAT : C_WAT + 128]
        WfrT_s = p16[:, C_WFRT : C_WFRT + 128]
        WfkT_s = p16[:, C_WFKT : C_WFKT + 128]
        MkT_s = p16[:, C_MKT : C_MKT + M]
        Mv0T_s = p16[:, C_MV0 : C_MV0 + M]
        WpT_s = p16[:, C_WPT : C_WPT + 1]
        one_s = p16[:, C_ONE : C_ONE + 1]
        be_s = p32[:, 0:1]
        ba_s = p32[:, 1:2]
        bf_s = p32[:, 2:3]
        bp_s = p32[:1, 3:4]

        eT = big.tile([D, BT], F16)      # sigmoid(We v + be), packed (b,t)
        aT = big.tile([D, BT], F16)
        wS = big.tile([M, BT], F16)      # softmax weights, packed (b,t)
        expw = big.tile([M, BT], F16)
        rz = big.tile([1, BT], F16)
        reads = big.tile([D, BT], F16)
        fT = big.tile([D, BT], F16)
        pS = big.tile([1, BT], F32)

        psB = ctx.enter_context(tc.tile_pool(name="psB", bufs=1, space="PSUM"))
        psW = ctx.enter_context(tc.tile_pool(name="psW", bufs=2, space="PSUM"))
        raccs = []
        for b in range(BL):
            r_ = psW.tile([D, T], F32, tag=f"racc{b}", bufs=1)
            raccs.append(r_)

        # ---- phase A ----
        if True:
            for b in range(BL):
                c = slice(b * T, (b + 1) * T)
                pe = psB.tile([D, 2 * T], F32, tag="wbc", bufs=2)[:, :T]
                nc.tensor.matmul(pe, WeT_s, vT_s[:, c], start=True, stop=True)
                nc.scalar.activation(eT[:, c], pe[:], ACT.Sigmoid, bias=be_s)

                pa = psB.tile([D, 2 * T], F32, tag="wbc", bufs=2)[:, :T]
                nc.tensor.matmul(pa, WaT_s, vT_s[:, c], start=True, stop=True)
                nc.scalar.activation(aT[:, c], pa[:], ACT.Tanh, bias=ba_s)

                pw = psB.tile([D, 2 * T], F32, tag="wbc", bufs=2)[:, :T]
                nc.tensor.matmul(pw[:M, :], MkT_s, kT_s[:, c], start=True, stop=True)
                # logits tiny (|x| < ~1): exp cannot overflow, skip max-sub
                nc.scalar.activation(expw[:, c], pw[:M, :], ACT.Exp)

                pz = psB.tile([D, 2 * T], F32, tag="wbc", bufs=2)[:, :T]
                nc.tensor.matmul(pz[:1, :], one_s[:M, :], expw[:, c],
                                 start=True, stop=True)
                with nc.allow_low_precision(reason="1/Z in [0.007,0.06], fp16 ok"):
                    nc.vector.reciprocal(rz[:, c], pz[:1, :])

                # broadcast 1/Z over 50 partitions: outer product ones x rz
                o1 = one_s[:1, :]
                o1b = bass.AP(o1.tensor, o1.offset, [list(o1.ap[0]), [0, M]])
                pzb = psB.tile([D, 2 * T], F32, tag="wbc", bufs=2)[:, :T]
                nc.tensor.matmul(pzb[:M, :], o1b, rz[:, c], start=True, stop=True)
                nc.vector.tensor_mul(wS[:, c], expw[:, c], pzb[:M, :])

        # ---- phase B ----
        if True:
            for m in range(M):
                # PE: broadcast w[m] -> [D, BT] via selector lhsT, in 2 halves
                col = iden_s[:M, m : m + 1]
                selT = bass.AP(col.tensor, col.offset, [list(col.ap[0]), [0, D]])
                wbc16 = sp.tile([D, BT], F16, tag="wbc16", bufs=6)
                for h in range(2):
                    hc = slice(h * (BT // 2), (h + 1) * (BT // 2))
                    ph = psB.tile([D, BT // 2], F32, tag="wbc", bufs=2)
                    for q in range(2):
                        qc = slice((2 * h + q) * T, (2 * h + q + 1) * T)
                        nc.tensor.matmul(
                            ph[:, q * T : (q + 1) * T], selT, wS[:, qc],
                            start=True, stop=True,
                        )
                    nc.scalar.activation(wbc16[:, hc], ph[:], ACT.Copy)

                # DVE: we = wbc*e (packed), Bt = wbc*a (strided out, fp16 2x)
                we = sp.tile([D, BT], F16, tag="we", bufs=4)
                nc.vector.tensor_mul(we[:], wbc16[:], eT[:])
                Bt = sp.tile([D, W], F16, tag="Bt", bufs=4)
                nc.vector.tensor_mul(
                    _ap3(Bt, 1, SEG, BL, 1, T), wbc16[:], aT[:]
                )
                # Act: boundary cols of Bt <- Mv0[:, m]
                mv = Mv0T_s[:, m : m + 1]
                mvb = bass.AP(mv.tensor, mv.offset, [list(mv.ap[0]), [0, BL]])
                nc.gpsimd.tensor_scalar(
                    _ap3(Bt, 0, SEG, BL, 1, 1), mvb, 1.0, None, mult
                )

                # Act: A = 1 - we (strided out; boundary cols pre-zeroed)
                At = sp.tile([D, W], F16, tag="At", bufs=4)
                if m < 4:
                    nc.vector.memset(_ap3(At, 0, SEG, BL, 1, 1), 0.0)
                nc.scalar.activation(
                    _ap3(At, 1, SEG, BL, 1, T), we[:], ACT.Identity,
                    bias=1.0, scale=-1.0,
                )

                # DVE: the scan (fp32 state internally)
                St = sp.tile([D, W], F16, tag="St", bufs=4)
                nc.vector.tensor_tensor_scan(St[:], At[:], Bt[:], 0.0, mult, add)

                # Pool: rt = wbc * S_pre  (S_pre = St shifted left by one col)
                rt = sp.tile([D, BT], F16, tag="rt", bufs=4)
                nc.gpsimd.tensor_mul(
                    rt[:], wbc16[:], _ap3(St, 0, SEG, BL, 1, T)
                )

                # PE: accumulate reads over m
                for b in range(BL):
                    c = slice(b * T, (b + 1) * T)
                    nc.tensor.matmul(
                        raccs[b], iden_s, rt[:, c],
                        start=(m == 0), stop=(m == M - 1),
                    )

            for b in range(BL):
                c = slice(b * T, (b + 1) * T)
                nc.gpsimd.tensor_scalar(reads[:, c], raccs[b][:], 1.0, None, mult)

        # ---- phase C ----
        if True:
            for b in range(BL):
                c = slice(b * T, (b + 1) * T)
                pf = psB.tile([D, 2 * T], F32, tag="wbc", bufs=2)[:, :T]
                nc.tensor.matmul(pf, WfrT_s, reads[:, c], start=True, stop=False)
                nc.tensor.matmul(pf, WfkT_s, kT_s[:, c], start=False, stop=True)
                nc.scalar.activation(fT[:, c], pf[:], ACT.Tanh, bias=bf_s)

                pp = psB.tile([D, 2 * T], F32, tag="wbc", bufs=2)[:, :T]
                nc.tensor.matmul(pp[:1, :], WpT_s, fT[:, c], start=True, stop=True)
                nc.scalar.activation(pS[:, c], pp[:1, :], ACT.Sigmoid, bias=bp_s)

        nc.sync.dma_start(out[:], pS[:])

    nc.compile()
    return nc


def _prep(q, r, Ek, Ev, Mk, Mv0, We, be, Wa, ba, Wf, bf, Wp, bp):
    q = np.asarray(q)
    r = np.asarray(r)
    mask = (r != 2).astype(np.int32)
    x = (q + NQ * r) * mask
    k = np.asarray(Ek)[q]            # [B, T, D] f32
    v = np.asarray(Ev)[x]

    prm16 = np.zeros((D, NP16), np.float16)
    prm16[:, C_IDEN : C_IDEN + 128] = np.eye(D, dtype=np.float16)
    prm16[:, C_WET : C_WET + 128] = np.asarray(We).T
    prm16[:, C_WAT : C_WAT + 128] = np.asarray(Wa).T
    prm16[:, C_WFRT : C_WFRT + 128] = np.asarray(Wf)[:, :D].T
    prm16[:, C_WFKT : C_WFKT + 128] = np.asarray(Wf)[:, D:].T
    prm16[:, C_MKT : C_MKT + M] = np.asarray(Mk).T
    prm16[:, C_MV0 : C_MV0 + M] = np.asarray(Mv0).T
    prm16[:, C_WPT] = np.asarray(Wp).ravel()
    prm16[:, C_ONE] = 1.0

    prm32 = np.zeros((D, 4), np.float32)
    prm32[:, 0] = np.asarray(be).ravel()
    prm32[:, 1] = np.asarray(ba).ravel()
    prm32[:, 2] = np.asarray(bf).ravel()
    prm32[0, 3] = np.asarray(bp).ravel()[0]
    shared = {"prm16": prm16, "prm32": prm32}

    in_maps = []
    for cidx in range(NCORES):
        sl = slice(cidx * BL, (cidx + 1) * BL)
        m = dict(shared)
        m["kT"] = np.ascontiguousarray(
            k[sl].transpose(2, 0, 1).reshape(D, BT)
        ).astype(np.float16)
        m["vT"] = np.ascontiguousarray(
            v[sl].transpose(2, 0, 1).reshape(D, BT)
        ).astype(np.float16)
        in_maps.append(m)
    return in_maps


def kernel(**inputs):
    if "nc" not in _CACHE:
        _CACHE["nc"] = _build()
    nc = _CACHE["nc"]
    in_maps = _prep(**inputs)
    res = run_bass_kernel_spmd(nc, in_maps, core_ids=list(range(NCORES)))
    outs = []
    for cidx in range(NCORES):
        outs.append(res.results[cidx]["out"].reshape(BL, T))
    return np.concatenate(outs, axis=0).astype(np.float32)


# revision 23
# speedup vs baseline: 1.0907x; 1.0907x over previous
"""DKVMN knowledge-tracing model on 8 Trainium2 NeuronCores — v2.

Sharding: data-parallel over batch (B=32 -> 4 rows/core); params replicated.

Per-core algorithm (BL=4, T=512, D=128, M=50), fp16 data / f32 accumulation:
  phase A: e = sigmoid(We v), a = tanh(Wa v), w = softmax_m(k @ Mk^T)
           (k/v arrive pre-gathered + transposed as fp16 [D, BL*T] from host)
  phase B (per m): one fused scan over all 4 batch rows using a
           W = BL*(T+1) = 2052-wide layout with a reset column per row
           (A=0, B=Mv0 at col b*513 restarts the recurrence at Mv0).
           Engine split per m:
             PE:   broadcast w[m] to 128 partitions (4 matmuls -> PSUM halves)
                   + 4 accumulate matmuls for reads
             Act:  PSUM->SBUF fp16 copy of the broadcast; A = 1 - w*e
                   (activation scale=-1 bias=1); Mv0 boundary patch
             DVE:  we = wbc*e, Bt = wbc*a (fp16 2x); the tensor_tensor_scan
             Pool: rt = wbc*S_pre via tensor_mul (gpsimd ucode)
  phase C: f = tanh(Wf [reads;k]); p = sigmoid(Wp f)
"""

import numpy as np
from contextlib import ExitStack

import concourse.bass as bass
import concourse.mybir as mybir
from concourse import tile
from concourse.bass_utils import run_bass_kernel_spmd
from concourse import bacc

B, T, D, M, NQ = 32, 512, 128, 50, 1000
NCORES = 8
BL = B // NCORES          # 4 batch rows per core
BT = BL * T               # 2048
W = BL * (T + 1)          # 2052: per-row segment of 513 (reset col + 512)
SEG = T + 1               # 513
F32 = mybir.dt.float32
F16 = mybir.dt.float16

# fp16 param pack layout (columns of prm16 [D, NP16])
C_IDEN = 0                # [D, 128] identity (racc lhsT; cols 0..49 also sel_m)
C_WET = 128               # We^T
C_WAT = 256               # Wa^T
C_WFRT = 384              # Wf[:, :D]^T
C_WFKT = 512              # Wf[:, D:]^T
C_MKT = 640               # Mk^T   [D, 50]
C_MV0 = 690               # Mv0^T  [D, 50]
C_WPT = 740               # Wp^T   [D, 1]
C_ONE = 741               # ones   [D, 1]
NP16 = 742

_CACHE = {}


def _ap3(t, col, s1, n1, s2, n2):
    """3-dim AP into tile t: partitions x [s1,n1] x [s2,n2], at column col."""
    base = t[:, col : col + 1]
    return bass.AP(base.tensor, base.offset, [list(base.ap[0]), [s1, n1], [s2, n2]])


def _build():
    nc = bacc.Bacc("TRN2", target_bir_lowering=False)

    kT = nc.dram_tensor("kT", [D, BT], F16, kind="ExternalInput")
    vT = nc.dram_tensor("vT", [D, BT], F16, kind="ExternalInput")
    prm16 = nc.dram_tensor("prm16", [D, NP16], F16, kind="ExternalInput")
    prm32 = nc.dram_tensor("prm32", [D, 4], F32, kind="ExternalInput")
    out = nc.dram_tensor("out", [1, BT], F32, kind="ExternalOutput")

    mult = mybir.AluOpType.mult
    add = mybir.AluOpType.add
    ACT = mybir.ActivationFunctionType

    with tile.TileContext(nc) as tc, ExitStack() as ctx:
        const = ctx.enter_context(tc.tile_pool(name="const", bufs=1))
        big = ctx.enter_context(tc.tile_pool(name="big", bufs=1))
        sp = ctx.enter_context(tc.tile_pool(name="sp", bufs=3))

        p16 = const.tile_from(prm16[:])
        kT_s = const.tile_from(kT[:])
        p32 = const.tile_from(prm32[:])
        vT_s = const.tile_from(vT[:])
        iden_s = p16[:, C_IDEN : C_IDEN + 128]
        WeT_s = p16[:, C_WET : C_WET + 128]
        WaT_s = p16[:, C_WAT : C_WAT + 128]
        WfrT_s = p16[:, C_WFRT : C_WFRT + 128]
        WfkT_s = p16[:, C_WFKT : C_WFKT + 128]
        MkT_s = p16[:, C_MKT : C_MKT + M]
        Mv0T_s = p16[:, C_MV0 : C_MV0 + M]
        WpT_s = p16[:, C_WPT : C_WPT + 1]
        one_s = p16[:, C_ONE : C_ONE + 1]
        be_s = p32[:, 0:1]
        ba_s = p32[:, 1:2]
        bf_s = p32[:, 2:3]
        bp_s = p32[:1, 3:4]

        warm = big.tile([1, 8], F32)
        nc.vector.memset(warm[:], 0.0)
        nc.scalar.activation(warm[:], warm[:], ACT.Exp)

        eT = big.tile([D, BT], F16)      # sigmoid(We v + be), packed (b,t)
        aT = big.tile([D, BT], F16)
        wS = big.tile([M, BT], F16)      # softmax weights, packed (b,t)
        expw = big.tile([M, BT], F16)
        rz = big.tile([1, BT], F16)
        reads = big.tile([D, BT], F16)
        fT = big.tile([D, BT], F16)
        pS = big.tile([1, BT], F32)

        psB = ctx.enter_context(tc.tile_pool(name="psB", bufs=1, space="PSUM"))
        psW = ctx.enter_context(tc.tile_pool(name="psW", bufs=2, space="PSUM"))
        raccs = []
        for b in range(BL):
            r_ = psW.tile([D, T], F32, tag=f"racc{b}", bufs=1)
            raccs.append(r_)

        # ---- phase A ----
        # Stage-major, softmax chain first: wS gates phase B's broadcasts,
        # so its chain (mm->exp->mm->recip->mm->mul) runs before e/a.
        if True:
            o1 = one_s[:1, :]
            o1b = bass.AP(o1.tensor, o1.offset, [list(o1.ap[0]), [0, M]])
            cs = [slice(b * T, (b + 1) * T) for b in range(BL)]
            # pw/pz/pzb park in racc[b] (disjoint partition ranges or
            # naturally serialized; racc proper resets at the m=0 matmul)
            for b in range(BL):
                pw = raccs[b][:M, :]
                nc.tensor.matmul(pw, MkT_s, kT_s[:, cs[b]], start=True, stop=True)
                # logits tiny (|x| < ~1): exp cannot overflow, skip max-sub
                nc.scalar.activation(expw[:, cs[b]], pw[:], ACT.Exp)
                pz = raccs[b][64:65, :]
                nc.tensor.matmul(pz, one_s[:M, :], expw[:, cs[b]],
                                 start=True, stop=True)
                with nc.allow_low_precision(reason="1/Z in [0.007,0.06], fp16 ok"):
                    nc.vector.reciprocal(rz[:, cs[b]], pz[:])
                pzb = raccs[b][:M, :]
                nc.tensor.matmul(pzb, o1b, rz[:, cs[b]], start=True, stop=True)
                nc.vector.tensor_mul(wS[:, cs[b]], expw[:, cs[b]], pzb[:])
            # e/a: pack two b-rows per [D, 2T] psum tile, one act per pair
            for h in range(2):
                pE = psB.tile([D, 2 * T], F32, tag="wbc", name="psbe", bufs=2)
                for q in range(2):
                    b = 2 * h + q
                    nc.tensor.matmul(pE[:, q * T : (q + 1) * T], WeT_s,
                                     vT_s[:, cs[b]], start=True, stop=True)
                nc.scalar.activation(eT[:, 2 * h * T : (2 * h + 2) * T], pE[:],
                                     ACT.Sigmoid, bias=be_s)
                pA = psB.tile([D, 2 * T], F32, tag="wbc", name="psba", bufs=2)
                for q in range(2):
                    b = 2 * h + q
                    nc.tensor.matmul(pA[:, q * T : (q + 1) * T], WaT_s,
                                     vT_s[:, cs[b]], start=True, stop=True)
                nc.scalar.activation(aT[:, 2 * h * T : (2 * h + 2) * T], pA[:],
                                     ACT.Tanh, bias=ba_s)

        # ---- phase B ----
        if True:
            for m in range(M):
                # PE: broadcast w[m] -> [D, BT] via selector lhsT, in 2 halves
                col = iden_s[:M, m : m + 1]
                selT = bass.AP(col.tensor, col.offset, [list(col.ap[0]), [0, D]])
                wbc16 = sp.tile([D, BT], F16, tag="wbc16", bufs=8)
                for h in range(2):
                    hc = slice(h * (BT // 2), (h + 1) * (BT // 2))
                    ph = psB.tile([D, BT // 2], F32, tag="wbc", bufs=2)
                    for q in range(2):
                        qc = slice((2 * h + q) * T, (2 * h + q + 1) * T)
                        nc.tensor.matmul(
                            ph[:, q * T : (q + 1) * T], selT, wS[:, qc],
                            start=True, stop=True,
                        )
                    nc.scalar.activation(wbc16[:, hc], ph[:], ACT.Copy)

                # DVE: we = wbc*e (packed), Bt = wbc*a (strided out, fp16 2x)
                we = sp.tile([D, BT], F16, tag="we", bufs=6)
                nc.vector.tensor_mul(we[:], wbc16[:], eT[:])
                Bt = sp.tile([D, W], F16, tag="Bt", bufs=6)
                nc.vector.tensor_mul(
                    _ap3(Bt, 1, SEG, BL, 1, T), wbc16[:], aT[:]
                )
                # Act: boundary cols of Bt <- Mv0[:, m]
                mv = Mv0T_s[:, m : m + 1]
                mvb = bass.AP(mv.tensor, mv.offset, [list(mv.ap[0]), [0, BL]])
                nc.scalar.activation(_ap3(Bt, 0, SEG, BL, 1, 1), mvb, ACT.Copy)

                # Act: A = 1 - we (strided out; boundary cols pre-zeroed)
                At = sp.tile([D, W], F16, tag="At", bufs=6)
                if m < 6:
                    nc.vector.memset(_ap3(At, 0, SEG, BL, 1, 1), 0.0)
                nc.scalar.activation(
                    _ap3(At, 1, SEG, BL, 1, T), we[:], ACT.Identity,
                    bias=1.0, scale=-1.0,
                )

                # DVE: the scan (fp32 state internally)
                St = sp.tile([D, W], F16, tag="St", bufs=6)
                nc.vector.tensor_tensor_scan(St[:], At[:], Bt[:], 0.0, mult, add)

                # Pool: rt = wbc * S_pre  (S_pre = St shifted left by one col)
                rt = sp.tile([D, BT], F16, tag="rt", bufs=6)
                nc.gpsimd.tensor_mul(
                    rt[:], wbc16[:], _ap3(St, 0, SEG, BL, 1, T)
                )

                # PE: accumulate reads over m
                for b in range(BL):
                    c = slice(b * T, (b + 1) * T)
                    nc.tensor.matmul(
                        raccs[b], iden_s, rt[:, c],
                        start=(m == 0), stop=(m == M - 1),
                    )

            for b in range(BL):
                c = slice(b * T, (b + 1) * T)
                nc.gpsimd.tensor_scalar(reads[:, c], raccs[b][:], 1.0, None, mult)

        # ---- phase C ----
        if True:
            for b in range(BL):
                c = slice(b * T, (b + 1) * T)
                pf = psB.tile([D, 2 * T], F32, tag="wbc", bufs=2)[:, :T]
                nc.tensor.matmul(pf, WfrT_s, reads[:, c], start=True, stop=False)
                nc.tensor.matmul(pf, WfkT_s, kT_s[:, c], start=False, stop=True)
                nc.scalar.activation(fT[:, c], pf[:], ACT.Tanh, bias=bf_s)

                pp = psB.tile([D, 2 * T], F32, tag="wbc", bufs=2)[:, :T]
                nc.tensor.matmul(pp[:1, :], WpT_s, fT[:, c], start=True, stop=True)
                nc.scalar.activation(pS[:, c], pp[:1, :], ACT.Sigmoid, bias=bp_s)

        nc.sync.dma_start(out[:], pS[:])

    nc.compile()
    return nc


def _prep(q, r, Ek, Ev, Mk, Mv0, We, be, Wa, ba, Wf, bf, Wp, bp):
    q = np.asarray(q)
    r = np.asarray(r)
    mask = (r != 2).astype(np.int32)
    x = (q + NQ * r) * mask
    k = np.asarray(Ek)[q]            # [B, T, D] f32
    v = np.asarray(Ev)[x]

    prm16 = np.zeros((D, NP16), np.float16)
    prm16[:, C_IDEN : C_IDEN + 128] = np.eye(D, dtype=np.float16)
    prm16[:, C_WET : C_WET + 128] = np.asarray(We).T
    prm16[:, C_WAT : C_WAT + 128] = np.asarray(Wa).T
    prm16[:, C_WFRT : C_WFRT + 128] = np.asarray(Wf)[:, :D].T
    prm16[:, C_WFKT : C_WFKT + 128] = np.asarray(Wf)[:, D:].T
    prm16[:, C_MKT : C_MKT + M] = np.asarray(Mk).T
    prm16[:, C_MV0 : C_MV0 + M] = np.asarray(Mv0).T
    prm16[:, C_WPT] = np.asarray(Wp).ravel()
    prm16[:, C_ONE] = 1.0

    prm32 = np.zeros((D, 4), np.float32)
    prm32[:, 0] = np.asarray(be).ravel()
    prm32[:, 1] = np.asarray(ba).ravel()
    prm32[:, 2] = np.asarray(bf).ravel()
    prm32[0, 3] = np.asarray(bp).ravel()[0]
    shared = {"prm16": prm16, "prm32": prm32}

    in_maps = []
    for cidx in range(NCORES):
        sl = slice(cidx * BL, (cidx + 1) * BL)
        m = dict(shared)
        m["kT"] = np.ascontiguousarray(
            k[sl].transpose(2, 0, 1).reshape(D, BT)
        ).astype(np.float16)
        m["vT"] = np.ascontiguousarray(
            v[sl].transpose(2, 0, 1).reshape(D, BT)
        ).astype(np.float16)
        in_maps.append(m)
    return in_maps


def kernel(**inputs):
    if "nc" not in _CACHE:
        _CACHE["nc"] = _build()
    nc = _CACHE["nc"]
    in_maps = _prep(**inputs)
    res = run_bass_kernel_spmd(nc, in_maps, core_ids=list(range(NCORES)))
    outs = []
    for cidx in range(NCORES):
        outs.append(res.results[cidx]["out"].reshape(BL, T))
    return np.concatenate(outs, axis=0).astype(np.float32)


# revision 42
# speedup vs baseline: 1.1170x; 1.0242x over previous
"""DKVMN knowledge-tracing model on 8 Trainium2 NeuronCores — v2.

Sharding: data-parallel over batch (B=32 -> 4 rows/core); params replicated.

Per-core algorithm (BL=4, T=512, D=128, M=50), fp16 data / f32 accumulation:
  phase A: e = sigmoid(We v), a = tanh(Wa v), w = softmax_m(k @ Mk^T)
           (k/v arrive pre-gathered + transposed as fp16 [D, BL*T] from host)
  phase B (per m): one fused scan over all 4 batch rows using a
           W = BL*(T+1) = 2052-wide layout with a reset column per row
           (A=0, B=Mv0 at col b*513 restarts the recurrence at Mv0).
           Engine split per m:
             PE:   broadcast w[m] to 128 partitions (4 matmuls -> PSUM halves)
                   + 4 accumulate matmuls for reads
             Act:  PSUM->SBUF fp16 copy of the broadcast; A = 1 - w*e
                   (activation scale=-1 bias=1); Mv0 boundary patch
             DVE:  we = wbc*e, Bt = wbc*a (fp16 2x); the tensor_tensor_scan
             Pool: rt = wbc*S_pre via tensor_mul (gpsimd ucode)
  phase C: f = tanh(Wf [reads;k]); p = sigmoid(Wp f)
"""

import numpy as np
from contextlib import ExitStack

import concourse.bass as bass
import concourse.mybir as mybir
from concourse import tile
from concourse.bass_utils import run_bass_kernel_spmd
from concourse import bacc

B, T, D, M, NQ = 32, 512, 128, 50, 1000
NCORES = 8
BL = B // NCORES          # 4 batch rows per core
BT = BL * T               # 2048
W = BL * (T + 1)          # 2052: per-row segment of 513 (reset col + 512)
SEG = T + 1               # 513
F32 = mybir.dt.float32
F16 = mybir.dt.float16

# fp16 param pack layout (columns of prm16 [D, NP16])
C_IDEN = 0                # [D, 128] identity (racc lhsT; cols 0..49 also sel_m)
C_WET = 128               # We^T
C_WAT = 256               # Wa^T
C_WFRT = 384              # Wf[:, :D]^T
C_WFKT = 512              # Wf[:, D:]^T
C_MKT = 640               # Mk^T   [D, 50]
C_MV0 = 690               # Mv0^T  [D, 50]
C_WPT = 740               # Wp^T   [D, 1]
C_ONE = 741               # ones   [D, 1]
NP16 = 742

_CACHE = {}


def _ap3(t, col, s1, n1, s2, n2):
    """3-dim AP into tile t: partitions x [s1,n1] x [s2,n2], at column col."""
    base = t[:, col : col + 1]
    return bass.AP(base.tensor, base.offset, [list(base.ap[0]), [s1, n1], [s2, n2]])


def _build():
    nc = bacc.Bacc("TRN2", target_bir_lowering=False)

    kT = nc.dram_tensor("kT", [D, BT], F16, kind="ExternalInput")
    vT = nc.dram_tensor("vT", [D, BT], F16, kind="ExternalInput")
    prm16 = nc.dram_tensor("prm16", [D, NP16], F16, kind="ExternalInput")
    prm32 = nc.dram_tensor("prm32", [D, 4], F32, kind="ExternalInput")
    out = nc.dram_tensor("out", [1, BT], F32, kind="ExternalOutput")

    mult = mybir.AluOpType.mult
    add = mybir.AluOpType.add
    ACT = mybir.ActivationFunctionType

    with tile.TileContext(nc) as tc, ExitStack() as ctx:
        const = ctx.enter_context(tc.tile_pool(name="const", bufs=1))
        big = ctx.enter_context(tc.tile_pool(name="big", bufs=1))
        sp = ctx.enter_context(tc.tile_pool(name="sp", bufs=3))

        p16 = const.tile_from(prm16[:])
        kT_s = const.tile([D, BT], F16, name="kT_s")
        nc.sync.dma_start(kT_s[:, 0:T], kT[:, 0:T])
        p32 = const.tile_from(prm32[:])
        nc.sync.dma_start(kT_s[:, T:BT], kT[:, T:BT])
        vT_s = const.tile_from(vT[:])
        iden_s = p16[:, C_IDEN : C_IDEN + 128]
        WeT_s = p16[:, C_WET : C_WET + 128]
        WaT_s = p16[:, C_WAT : C_WAT + 128]
        WfrT_s = p16[:, C_WFRT : C_WFRT + 128]
        WfkT_s = p16[:, C_WFKT : C_WFKT + 128]
        MkT_s = p16[:, C_MKT : C_MKT + M]
        Mv0T_s = p16[:, C_MV0 : C_MV0 + M]
        WpT_s = p16[:, C_WPT : C_WPT + 1]
        one_s = p16[:, C_ONE : C_ONE + 1]
        be_s = p32[:, 0:1]
        ba_s = p32[:, 1:2]
        bf_s = p32[:, 2:3]
        bp_s = p32[:1, 3:4]

        warm = big.tile([1, 8], F32)
        nc.vector.memset(warm[:], 0.0)
        nc.scalar.activation(warm[:], warm[:], ACT.Exp)

        eT = big.tile([D, BT], F16)      # sigmoid(We v + be), packed (b,t)
        aT = big.tile([D, BT], F16)
        wS = big.tile([M, BT], F16)      # softmax weights, packed (b,t)
        expw = big.tile([M, BT], F16)
        rz = big.tile([1, BT], F16)
        reads = big.tile([D, BT], F16)
        fT = big.tile([D, BT], F16)
        pS = big.tile([1, BT], F32)

        psB = ctx.enter_context(tc.tile_pool(name="psB", bufs=1, space="PSUM"))
        psW = ctx.enter_context(tc.tile_pool(name="psW", bufs=2, space="PSUM"))
        raccs = []
        for b in range(BL):
            r_ = psW.tile([D, T], F32, tag=f"racc{b}", bufs=1)
            raccs.append(r_)

        # ---- phase A ----
        # Stage-major, softmax chain first: wS gates phase B's broadcasts,
        # so its chain (mm->exp->mm->recip->mm->mul) runs before e/a.
        if True:
            o1 = one_s[:1, :]
            o1b = bass.AP(o1.tensor, o1.offset, [list(o1.ap[0]), [0, M]])
            cs = [slice(b * T, (b + 1) * T) for b in range(BL)]
            # pw/pz/pzb park in racc[b] (disjoint partition ranges or
            # naturally serialized; racc proper resets at the m=0 matmul)
            for b in range(BL):
                pw = raccs[b][:M, :]
                nc.tensor.matmul(pw, MkT_s, kT_s[:, cs[b]], start=True, stop=True)
                # logits tiny (|x| < ~1): exp cannot overflow, skip max-sub
                nc.scalar.activation(expw[:, cs[b]], pw[:], ACT.Exp)
                pz = raccs[b][64:65, :]
                nc.tensor.matmul(pz, one_s[:M, :], expw[:, cs[b]],
                                 start=True, stop=True)
                with nc.allow_low_precision(reason="1/Z in [0.007,0.06], fp16 ok"):
                    nc.vector.reciprocal(rz[:, cs[b]], pz[:])
                pzb = raccs[b][:M, :]
                nc.tensor.matmul(pzb, o1b, rz[:, cs[b]], start=True, stop=True)
                nc.vector.tensor_mul(wS[:, cs[b]], expw[:, cs[b]], pzb[:])
            # e/a: pack two b-rows per [D, 2T] psum tile, one act per pair
            for h in range(2):
                pE = psB.tile([D, 2 * T], F32, tag="wbc", name="psbe", bufs=2)
                for q in range(2):
                    b = 2 * h + q
                    nc.tensor.matmul(pE[:, q * T : (q + 1) * T], WeT_s,
                                     vT_s[:, cs[b]], start=True, stop=True)
                nc.scalar.activation(eT[:, 2 * h * T : (2 * h + 2) * T], pE[:],
                                     ACT.Sigmoid, bias=be_s)
                pA = psB.tile([D, 2 * T], F32, tag="wbc", name="psba", bufs=2)
                for q in range(2):
                    b = 2 * h + q
                    nc.tensor.matmul(pA[:, q * T : (q + 1) * T], WaT_s,
                                     vT_s[:, cs[b]], start=True, stop=True)
                nc.scalar.activation(aT[:, 2 * h * T : (2 * h + 2) * T], pA[:],
                                     ACT.Tanh, bias=ba_s)

        # ---- phase B ----
        if True:
            for m in range(M):
                # PE: broadcast w[m] -> [D, BT] via selector lhsT, in 2 halves
                col = iden_s[:M, m : m + 1]
                selT = bass.AP(col.tensor, col.offset, [list(col.ap[0]), [0, D]])
                wbc16 = sp.tile([D, BT], F16, tag="wbc16", bufs=10)
                for h in range(2):
                    hc = slice(h * (BT // 2), (h + 1) * (BT // 2))
                    ph = psB.tile([D, BT // 2], F32, tag="wbc", bufs=2)
                    for q in range(2):
                        qc = slice((2 * h + q) * T, (2 * h + q + 1) * T)
                        nc.tensor.matmul(
                            ph[:, q * T : (q + 1) * T], selT, wS[:, qc],
                            start=True, stop=True,
                        )
                    nc.scalar.activation(wbc16[:, hc], ph[:], ACT.Copy)

                # DVE: we = wbc*e (packed), Bt = wbc*a (strided out, fp16 2x)
                we = sp.tile([D, BT], F16, tag="we", bufs=6)
                if m < 2:
                    # pipeline fill: halves start as soon as each copy lands
                    for h in range(2):
                        hc = slice(h * (BT // 2), (h + 1) * (BT // 2))
                        nc.vector.tensor_mul(we[:, hc], wbc16[:, hc], eT[:, hc])
                else:
                    nc.vector.tensor_mul(we[:], wbc16[:], eT[:])
                Bt = sp.tile([D, W], F16, tag="Bt", bufs=6)
                if m < 3:
                    for h in range(2):
                        hc = slice(h * (BT // 2), (h + 1) * (BT // 2))
                        nc.vector.tensor_mul(
                            _ap3(Bt, h * 2 * SEG + 1, SEG, 2, 1, T),
                            wbc16[:, hc], aT[:, hc],
                        )
                else:
                    nc.vector.tensor_mul(
                        _ap3(Bt, 1, SEG, BL, 1, T), wbc16[:], aT[:]
                    )
                # Act: boundary cols of Bt <- Mv0[:, m]
                mv = Mv0T_s[:, m : m + 1]
                mvb = bass.AP(mv.tensor, mv.offset, [list(mv.ap[0]), [0, BL]])
                nc.scalar.activation(_ap3(Bt, 0, SEG, BL, 1, 1), mvb, ACT.Copy)

                # Act: A = 1 - we (strided out; boundary cols pre-zeroed)
                At = sp.tile([D, W], F16, tag="At", bufs=6)
                if m < 6:
                    nc.vector.memset(_ap3(At, 0, SEG, BL, 1, 1), 0.0)
                if m < 2:
                    for h in range(2):
                        hc = slice(h * (BT // 2), (h + 1) * (BT // 2))
                        nc.scalar.activation(
                            _ap3(At, h * 2 * SEG + 1, SEG, 2, 1, T),
                            we[:, hc], ACT.Identity, bias=1.0, scale=-1.0,
                        )
                else:
                    nc.scalar.activation(
                        _ap3(At, 1, SEG, BL, 1, T), we[:], ACT.Identity,
                        bias=1.0, scale=-1.0,
                    )

                # DVE: the scan (fp32 state internally); the last one in
                # halves so the drain's per-b rt chain starts earlier
                St = sp.tile([D, W], F16, tag="St", bufs=6)
                if m == M - 1:
                    for h in range(2):
                        hs = slice(h * 2 * SEG, (h + 1) * 2 * SEG)
                        nc.vector.tensor_tensor_scan(
                            St[:, hs], At[:, hs], Bt[:, hs], 0.0, mult, add
                        )
                else:
                    nc.vector.tensor_tensor_scan(St[:], At[:], Bt[:], 0.0, mult, add)

                # Pool: rt = wbc * S_pre  (S_pre = St shifted left by one col)
                rt = sp.tile([D, BT], F16, tag="rt", bufs=6)
                nc.gpsimd.tensor_mul(
                    rt[:], wbc16[:], _ap3(St, 0, SEG, BL, 1, T)
                )

                # PE: accumulate reads over m
                for b in range(BL):
                    c = slice(b * T, (b + 1) * T)
                    nc.tensor.matmul(
                        raccs[b], iden_s, rt[:, c],
                        start=(m == 0), stop=(m == M - 1),
                    )

            for b in range(BL):
                c = slice(b * T, (b + 1) * T)
                nc.gpsimd.tensor_scalar(reads[:, c], raccs[b][:], 1.0, None, mult)

        # ---- phase C ----
        if True:
            for b in range(BL):
                c = slice(b * T, (b + 1) * T)
                pf = psB.tile([D, 2 * T], F32, tag="wbc", bufs=2)[:, :T]
                nc.tensor.matmul(pf, WfrT_s, reads[:, c], start=True, stop=False)
                nc.tensor.matmul(pf, WfkT_s, kT_s[:, c], start=False, stop=True)
                nc.scalar.activation(fT[:, c], pf[:], ACT.Tanh, bias=bf_s)

                pp = psB.tile([D, 2 * T], F32, tag="wbc", bufs=2)[:, :T]
                nc.tensor.matmul(pp[:1, :], WpT_s, fT[:, c], start=True, stop=True)
                nc.scalar.activation(pS[:, c], pp[:1, :], ACT.Sigmoid, bias=bp_s)

        nc.sync.dma_start(out[:], pS[:])

    nc.compile()
    return nc


def _prep(q, r, Ek, Ev, Mk, Mv0, We, be, Wa, ba, Wf, bf, Wp, bp):
    q = np.asarray(q)
    r = np.asarray(r)
    mask = (r != 2).astype(np.int32)
    x = (q + NQ * r) * mask
    k = np.asarray(Ek)[q]            # [B, T, D] f32
    v = np.asarray(Ev)[x]

    prm16 = np.zeros((D, NP16), np.float16)
    prm16[:, C_IDEN : C_IDEN + 128] = np.eye(D, dtype=np.float16)
    prm16[:, C_WET : C_WET + 128] = np.asarray(We).T
    prm16[:, C_WAT : C_WAT + 128] = np.asarray(Wa).T
    prm16[:, C_WFRT : C_WFRT + 128] = np.asarray(Wf)[:, :D].T
    prm16[:, C_WFKT : C_WFKT + 128] = np.asarray(Wf)[:, D:].T
    prm16[:, C_MKT : C_MKT + M] = np.asarray(Mk).T
    prm16[:, C_MV0 : C_MV0 + M] = np.asarray(Mv0).T
    prm16[:, C_WPT] = np.asarray(Wp).ravel()
    prm16[:, C_ONE] = 1.0

    prm32 = np.zeros((D, 4), np.float32)
    prm32[:, 0] = np.asarray(be).ravel()
    prm32[:, 1] = np.asarray(ba).ravel()
    prm32[:, 2] = np.asarray(bf).ravel()
    prm32[0, 3] = np.asarray(bp).ravel()[0]
    shared = {"prm16": prm16, "prm32": prm32}

    in_maps = []
    for cidx in range(NCORES):
        sl = slice(cidx * BL, (cidx + 1) * BL)
        m = dict(shared)
        m["kT"] = np.ascontiguousarray(
            k[sl].transpose(2, 0, 1).reshape(D, BT)
        ).astype(np.float16)
        m["vT"] = np.ascontiguousarray(
            v[sl].transpose(2, 0, 1).reshape(D, BT)
        ).astype(np.float16)
        in_maps.append(m)
    return in_maps


def kernel(**inputs):
    if "nc" not in _CACHE:
        _CACHE["nc"] = _build()
    nc = _CACHE["nc"]
    in_maps = _prep(**inputs)
    res = run_bass_kernel_spmd(nc, in_maps, core_ids=list(range(NCORES)))
    outs = []
    for cidx in range(NCORES):
        outs.append(res.results[cidx]["out"].reshape(BL, T))
    return np.concatenate(outs, axis=0).astype(np.float32)


# revision 44
# speedup vs baseline: 1.1187x; 1.0015x over previous
"""DKVMN knowledge-tracing model on 8 Trainium2 NeuronCores — v2.

Sharding: data-parallel over batch (B=32 -> 4 rows/core); params replicated.

Per-core algorithm (BL=4, T=512, D=128, M=50), fp16 data / f32 accumulation:
  phase A: e = sigmoid(We v), a = tanh(Wa v), w = softmax_m(k @ Mk^T)
           (k/v arrive pre-gathered + transposed as fp16 [D, BL*T] from host)
  phase B (per m): one fused scan over all 4 batch rows using a
           W = BL*(T+1) = 2052-wide layout with a reset column per row
           (A=0, B=Mv0 at col b*513 restarts the recurrence at Mv0).
           Engine split per m:
             PE:   broadcast w[m] to 128 partitions (4 matmuls -> PSUM halves)
                   + 4 accumulate matmuls for reads
             Act:  PSUM->SBUF fp16 copy of the broadcast; A = 1 - w*e
                   (activation scale=-1 bias=1); Mv0 boundary patch
             DVE:  we = wbc*e, Bt = wbc*a (fp16 2x); the tensor_tensor_scan
             Pool: rt = wbc*S_pre via tensor_mul (gpsimd ucode)
  phase C: f = tanh(Wf [reads;k]); p = sigmoid(Wp f)
"""

import numpy as np
from contextlib import ExitStack

import concourse.bass as bass
import concourse.mybir as mybir
from concourse import tile
from concourse.bass_utils import run_bass_kernel_spmd
from concourse import bacc

B, T, D, M, NQ = 32, 512, 128, 50, 1000
NCORES = 8
BL = B // NCORES          # 4 batch rows per core
BT = BL * T               # 2048
W = BL * (T + 1)          # 2052: per-row segment of 513 (reset col + 512)
SEG = T + 1               # 513
F32 = mybir.dt.float32
F16 = mybir.dt.float16

# fp16 param pack layout (columns of prm16 [D, NP16])
C_IDEN = 0                # [D, 128] identity (racc lhsT; cols 0..49 also sel_m)
C_WET = 128               # We^T
C_WAT = 256               # Wa^T
C_WFRT = 384              # Wf[:, :D]^T
C_WFKT = 512              # Wf[:, D:]^T
C_MKT = 640               # Mk^T   [D, 50]
C_MV0 = 690               # Mv0^T  [D, 50]
C_WPT = 740               # Wp^T   [D, 1]
C_ONE = 741               # ones   [D, 1]
NP16 = 742

_CACHE = {}


def _ap3(t, col, s1, n1, s2, n2):
    """3-dim AP into tile t: partitions x [s1,n1] x [s2,n2], at column col."""
    base = t[:, col : col + 1]
    return bass.AP(base.tensor, base.offset, [list(base.ap[0]), [s1, n1], [s2, n2]])


def _build():
    nc = bacc.Bacc("TRN2", target_bir_lowering=False)

    kT = nc.dram_tensor("kT", [D, BT], F16, kind="ExternalInput")
    vT = nc.dram_tensor("vT", [D, BT], F16, kind="ExternalInput")
    prm16 = nc.dram_tensor("prm16", [D, NP16], F16, kind="ExternalInput")
    prm32 = nc.dram_tensor("prm32", [D, 4], F32, kind="ExternalInput")
    out = nc.dram_tensor("out", [1, BT], F32, kind="ExternalOutput")

    mult = mybir.AluOpType.mult
    add = mybir.AluOpType.add
    ACT = mybir.ActivationFunctionType

    with tile.TileContext(nc) as tc, ExitStack() as ctx:
        const = ctx.enter_context(tc.tile_pool(name="const", bufs=1))
        big = ctx.enter_context(tc.tile_pool(name="big", bufs=1))
        sp = ctx.enter_context(tc.tile_pool(name="sp", bufs=3))

        p16 = const.tile_from(prm16[:])
        kT_s = const.tile([D, BT], F16, name="kT_s")
        nc.sync.dma_start(kT_s[:, 0:T], kT[:, 0:T])
        nc.sync.dma_start(kT_s[:, T:BT], kT[:, T:BT])
        vT_s = const.tile_from(vT[:])
        p32 = const.tile_from(prm32[:])
        iden_s = p16[:, C_IDEN : C_IDEN + 128]
        WeT_s = p16[:, C_WET : C_WET + 128]
        WaT_s = p16[:, C_WAT : C_WAT + 128]
        WfrT_s = p16[:, C_WFRT : C_WFRT + 128]
        WfkT_s = p16[:, C_WFKT : C_WFKT + 128]
        MkT_s = p16[:, C_MKT : C_MKT + M]
        Mv0T_s = p16[:, C_MV0 : C_MV0 + M]
        WpT_s = p16[:, C_WPT : C_WPT + 1]
        one_s = p16[:, C_ONE : C_ONE + 1]
        be_s = p32[:, 0:1]
        ba_s = p32[:, 1:2]
        bf_s = p32[:, 2:3]
        bp_s = p32[:1, 3:4]

        warm = big.tile([1, 8], F32)
        nc.vector.memset(warm[:], 0.0)
        nc.scalar.activation(warm[:], warm[:], ACT.Exp)

        eT = big.tile([D, BT], F16)      # sigmoid(We v + be), packed (b,t)
        aT = big.tile([D, BT], F16)
        wS = big.tile([M, BT], F16)      # softmax weights, packed (b,t)
        expw = big.tile([M, BT], F16)
        rz = big.tile([1, BT], F16)
        reads = big.tile([D, BT], F16)
        fT = big.tile([D, BT], F16)
        pS = big.tile([1, BT], F32)

        psB = ctx.enter_context(tc.tile_pool(name="psB", bufs=1, space="PSUM"))
        psW = ctx.enter_context(tc.tile_pool(name="psW", bufs=2, space="PSUM"))
        raccs = []
        for b in range(BL):
            r_ = psW.tile([D, T], F32, tag=f"racc{b}", bufs=1)
            raccs.append(r_)

        # ---- phase A ----
        # Stage-major, softmax chain first: wS gates phase B's broadcasts,
        # so its chain (mm->exp->mm->recip->mm->mul) runs before e/a.
        if True:
            o1 = one_s[:1, :]
            o1b = bass.AP(o1.tensor, o1.offset, [list(o1.ap[0]), [0, M]])
            cs = [slice(b * T, (b + 1) * T) for b in range(BL)]
            # pw/pz/pzb park in racc[b] (disjoint partition ranges or
            # naturally serialized; racc proper resets at the m=0 matmul)
            for b in range(BL):
                pw = raccs[b][:M, :]
                nc.tensor.matmul(pw, MkT_s, kT_s[:, cs[b]], start=True, stop=True)
                # logits tiny (|x| < ~1): exp cannot overflow, skip max-sub
                nc.scalar.activation(expw[:, cs[b]], pw[:], ACT.Exp)
                pz = raccs[b][64:65, :]
                nc.tensor.matmul(pz, one_s[:M, :], expw[:, cs[b]],
                                 start=True, stop=True)
                with nc.allow_low_precision(reason="1/Z in [0.007,0.06], fp16 ok"):
                    nc.vector.reciprocal(rz[:, cs[b]], pz[:])
                pzb = raccs[b][:M, :]
                nc.tensor.matmul(pzb, o1b, rz[:, cs[b]], start=True, stop=True)
                nc.vector.tensor_mul(wS[:, cs[b]], expw[:, cs[b]], pzb[:])
            # e/a: pack two b-rows per [D, 2T] psum tile, one act per pair
            for h in range(2):
                pE = psB.tile([D, 2 * T], F32, tag="wbc", name="psbe", bufs=2)
                for q in range(2):
                    b = 2 * h + q
                    nc.tensor.matmul(pE[:, q * T : (q + 1) * T], WeT_s,
                                     vT_s[:, cs[b]], start=True, stop=True)
                nc.scalar.activation(eT[:, 2 * h * T : (2 * h + 2) * T], pE[:],
                                     ACT.Sigmoid, bias=be_s)
                pA = psB.tile([D, 2 * T], F32, tag="wbc", name="psba", bufs=2)
                for q in range(2):
                    b = 2 * h + q
                    nc.tensor.matmul(pA[:, q * T : (q + 1) * T], WaT_s,
                                     vT_s[:, cs[b]], start=True, stop=True)
                nc.scalar.activation(aT[:, 2 * h * T : (2 * h + 2) * T], pA[:],
                                     ACT.Tanh, bias=ba_s)

        # ---- phase B ----
        if True:
            for m in range(M):
                # PE: broadcast w[m] -> [D, BT] via selector lhsT, in 2 halves
                col = iden_s[:M, m : m + 1]
                selT = bass.AP(col.tensor, col.offset, [list(col.ap[0]), [0, D]])
                wbc16 = sp.tile([D, BT], F16, tag="wbc16", bufs=10)
                for h in range(2):
                    hc = slice(h * (BT // 2), (h + 1) * (BT // 2))
                    ph = psB.tile([D, BT // 2], F32, tag="wbc", bufs=2)
                    for q in range(2):
                        qc = slice((2 * h + q) * T, (2 * h + q + 1) * T)
                        nc.tensor.matmul(
                            ph[:, q * T : (q + 1) * T], selT, wS[:, qc],
                            start=True, stop=True,
                        )
                    nc.scalar.activation(wbc16[:, hc], ph[:], ACT.Copy)

                # DVE: we = wbc*e (packed), Bt = wbc*a (strided out, fp16 2x)
                we = sp.tile([D, BT], F16, tag="we", bufs=6)
                if m < 2:
                    # pipeline fill: halves start as soon as each copy lands
                    for h in range(2):
                        hc = slice(h * (BT // 2), (h + 1) * (BT // 2))
                        nc.vector.tensor_mul(we[:, hc], wbc16[:, hc], eT[:, hc])
                else:
                    nc.vector.tensor_mul(we[:], wbc16[:], eT[:])
                Bt = sp.tile([D, W], F16, tag="Bt", bufs=6)
                if m < 3:
                    for h in range(2):
                        hc = slice(h * (BT // 2), (h + 1) * (BT // 2))
                        nc.vector.tensor_mul(
                            _ap3(Bt, h * 2 * SEG + 1, SEG, 2, 1, T),
                            wbc16[:, hc], aT[:, hc],
                        )
                else:
                    nc.vector.tensor_mul(
                        _ap3(Bt, 1, SEG, BL, 1, T), wbc16[:], aT[:]
                    )
                # Act: boundary cols of Bt <- Mv0[:, m]
                mv = Mv0T_s[:, m : m + 1]
                mvb = bass.AP(mv.tensor, mv.offset, [list(mv.ap[0]), [0, BL]])
                nc.scalar.activation(_ap3(Bt, 0, SEG, BL, 1, 1), mvb, ACT.Copy)

                # Act: A = 1 - we (strided out; boundary cols pre-zeroed)
                At = sp.tile([D, W], F16, tag="At", bufs=6)
                if m < 6:
                    nc.vector.memset(_ap3(At, 0, SEG, BL, 1, 1), 0.0)
                if m < 2:
                    for h in range(2):
                        hc = slice(h * (BT // 2), (h + 1) * (BT // 2))
                        nc.scalar.activation(
                            _ap3(At, h * 2 * SEG + 1, SEG, 2, 1, T),
                            we[:, hc], ACT.Identity, bias=1.0, scale=-1.0,
                        )
                else:
                    nc.scalar.activation(
                        _ap3(At, 1, SEG, BL, 1, T), we[:], ACT.Identity,
                        bias=1.0, scale=-1.0,
                    )

                # DVE: the scan (fp32 state internally); the last one in
                # halves so the drain's per-b rt chain starts earlier
                St = sp.tile([D, W], F16, tag="St", bufs=6)
                if m == M - 1:
                    for h in range(2):
                        hs = slice(h * 2 * SEG, (h + 1) * 2 * SEG)
                        nc.vector.tensor_tensor_scan(
                            St[:, hs], At[:, hs], Bt[:, hs], 0.0, mult, add
                        )
                else:
                    nc.vector.tensor_tensor_scan(St[:], At[:], Bt[:], 0.0, mult, add)

                # Pool: rt = wbc * S_pre  (S_pre = St shifted left by one col)
                rt = sp.tile([D, BT], F16, tag="rt", bufs=6)
                nc.gpsimd.tensor_mul(
                    rt[:], wbc16[:], _ap3(St, 0, SEG, BL, 1, T)
                )

                # PE: accumulate reads over m
                for b in range(BL):
                    c = slice(b * T, (b + 1) * T)
                    nc.tensor.matmul(
                        raccs[b], iden_s, rt[:, c],
                        start=(m == 0), stop=(m == M - 1),
                    )

            for b in range(BL):
                c = slice(b * T, (b + 1) * T)
                nc.gpsimd.tensor_scalar(reads[:, c], raccs[b][:], 1.0, None, mult)

        # ---- phase C ----
        if True:
            for b in range(BL):
                c = slice(b * T, (b + 1) * T)
                pf = psB.tile([D, 2 * T], F32, tag="wbc", bufs=2)[:, :T]
                nc.tensor.matmul(pf, WfrT_s, reads[:, c], start=True, stop=False)
                nc.tensor.matmul(pf, WfkT_s, kT_s[:, c], start=False, stop=True)
                nc.scalar.activation(fT[:, c], pf[:], ACT.Tanh, bias=bf_s)

                pp = psB.tile([D, 2 * T], F32, tag="wbc", bufs=2)[:, :T]
                nc.tensor.matmul(pp[:1, :], WpT_s, fT[:, c], start=True, stop=True)
                nc.scalar.activation(pS[:, c], pp[:1, :], ACT.Sigmoid, bias=bp_s)

        nc.sync.dma_start(out[:], pS[:])

    nc.compile()
    return nc


def _prep(q, r, Ek, Ev, Mk, Mv0, We, be, Wa, ba, Wf, bf, Wp, bp):
    q = np.asarray(q)
    r = np.asarray(r)
    mask = (r != 2).astype(np.int32)
    x = (q + NQ * r) * mask
    k = np.asarray(Ek)[q]            # [B, T, D] f32
    v = np.asarray(Ev)[x]

    prm16 = np.zeros((D, NP16), np.float16)
    prm16[:, C_IDEN : C_IDEN + 128] = np.eye(D, dtype=np.float16)
    prm16[:, C_WET : C_WET + 128] = np.asarray(We).T
    prm16[:, C_WAT : C_WAT + 128] = np.asarray(Wa).T
    prm16[:, C_WFRT : C_WFRT + 128] = np.asarray(Wf)[:, :D].T
    prm16[:, C_WFKT : C_WFKT + 128] = np.asarray(Wf)[:, D:].T
    prm16[:, C_MKT : C_MKT + M] = np.asarray(Mk).T
    prm16[:, C_MV0 : C_MV0 + M] = np.asarray(Mv0).T
    prm16[:, C_WPT] = np.asarray(Wp).ravel()
    prm16[:, C_ONE] = 1.0

    prm32 = np.zeros((D, 4), np.float32)
    prm32[:, 0] = np.asarray(be).ravel()
    prm32[:, 1] = np.asarray(ba).ravel()
    prm32[:, 2] = np.asarray(bf).ravel()
    prm32[0, 3] = np.asarray(bp).ravel()[0]
    shared = {"prm16": prm16, "prm32": prm32}

    in_maps = []
    for cidx in range(NCORES):
        sl = slice(cidx * BL, (cidx + 1) * BL)
        m = dict(shared)
        m["kT"] = np.ascontiguousarray(
            k[sl].transpose(2, 0, 1).reshape(D, BT)
        ).astype(np.float16)
        m["vT"] = np.ascontiguousarray(
            v[sl].transpose(2, 0, 1).reshape(D, BT)
        ).astype(np.float16)
        in_maps.append(m)
    return in_maps


def kernel(**inputs):
    if "nc" not in _CACHE:
        _CACHE["nc"] = _build()
    nc = _CACHE["nc"]
    in_maps = _prep(**inputs)
    res = run_bass_kernel_spmd(nc, in_maps, core_ids=list(range(NCORES)))
    outs = []
    for cidx in range(NCORES):
        outs.append(res.results[cidx]["out"].reshape(BL, T))
    return np.concatenate(outs, axis=0).astype(np.float32)
